# revision 47
# baseline (speedup 1.0000x reference)
"""Paged-attention GQA decode kernel for 8 Trainium2 NeuronCores.

Problem: B=16 sequences, H=32 query heads, KVH=8 KV heads (GQA group G=4),
D=128, paged KV cache of 65536 slots (block size 256, 16 blocks/seq,
max context 4096).

Sharding: tensor-parallel over KV heads — core c owns KV head c and the
4 query heads of its GQA group, for all 16 sequences.

Host-side prep (per core, plain numpy — this is the shard/relayout step):
  * scatter the new k/v rows into the cache view (reference step 1),
  * gather each sequence's context via its block table (reference step 2),
  * K transposed to [d, s] bf16 (the PE contracts over d); V slot-major
    per 128-slot chunk, [slot, d] fp8 e3m4 (4 mantissa bits; V error
    enters the output linearly and 1.3e-2 max-rel fits the 2e-2 gate,
    while fp8 K would not — exp amplifies score noise, measured 2.02e-2).
Rows past a sequence's context length are zeroed in V, so padded slots
contribute exactly 0 to the numerator; the host drops them from the
denominator sum (it knows the valid mask).

Device kernel (per core), per sequence, per 128-slot chunk:
  scoresT[s,g] = KT_chunk.T @ QT      (PE; 128-col bf16 stationary gets
                                       the automatic fast-weight-load)
  expT         = exp(scoresT * SCALE) (ACT, bf16 out; no max-subtraction
                                       — scores are ~N(0,1))
  ot[d,g]     += V_chunk.T @ expT     (PE; 128-col fp8 stationary → FWL,
                                       4 weights/cycle; only G=4 moving
                                       columns per chunk)
  ob[:, b]     = copy(ot)             (DVE evacuates PSUM)
and ships BOTH the un-normalized ot and the expT tensor to the host,
which computes den[g] = sum of valid expT rows and divides — softmax
normalization is a tiny host-side epilogue on [B,H] scalars.

DMA strategy: sequences are paired into 8 kt + 8 vt group-loads (a DMA's
HWDGE descriptor count is 128 either way, and the ring only holds ~3
DMAs' descriptors — bigger transfers keep more bytes in flight), all
issued up front: kt groups on the SP ring, vt groups on the ACT ring
ahead of the exps.  All KV tiles are SBUF-resident.  Measured bottleneck
is the KV-cache DMA traffic, as intended for this memory-bound regime.
"""

import ml_dtypes
import numpy as np

B, H, KVH, D = 16, 32, 8, 128
G = H // KVH  # 4
BLOCK_SIZE = 256
MAX_CTX = 4096
SCALE = 0.08838834764831845  # 1/sqrt(128)
NCORES = 8
CHUNK = 128
GROUP = 2  # sequences per load-DMA

KT_NP = ml_dtypes.bfloat16
VT_NP = ml_dtypes.float8_e3m4
Q_NP = ml_dtypes.bfloat16
ET_NP = ml_dtypes.bfloat16
KT_MYBIR = "bfloat16"
VT_MYBIR = "float8e3"
Q_MYBIR = "bfloat16"
ET_MYBIR = "bfloat16"

TRACE = False  # set by test harness to capture an NTFF profile
LAST_RESULT = None  # BassKernelResults of the most recent run (for the harness)

_nc_cache = {}


def _install_ntff_shim():
    """Register the NTFF profile hook concourse looks for under axon.

    The agent image's ``antenv`` lacks ``axon_hooks``; the ctypes hook
    implementation ships in ``trn_agent_boot`` — wire the two together.
    """
    import sys
    import types

    if "antenv.axon_hooks" in sys.modules:
        return
    try:
        import trn_agent_boot.trn_boot as tb

        hook = tb._ntff_profile_via_ctypes("/opt/axon/libaxon_pjrt.so")
    except Exception:
        return
    mod = types.ModuleType("antenv.axon_hooks")
    mod.get_axon_ntff_profile_hook = lambda: hook
    sys.modules["antenv.axon_hooks"] = mod


def _split_multi_waits(nc):
    """Legalize sync waits for this walrus build.

    The Tile scheduler attaches one wait per producer semaphore to an
    instruction (up to 4 here), but this walrus rejects more than 1 sync
    wait per instruction (2 on EventSemaphore).  Splitting the extras
    onto same-engine nops placed immediately before the instruction
    preserves semantics: engines execute their stream in order, so all
    waits still complete before the instruction runs.
    """
    import concourse.mybir as mybir

    n = 0
    for fn in nc.m.functions:
        for blk in fn.blocks:
            out = []
            changed = False
            for inst in blk.instructions:
                si = inst.sync_info
                cap = 2 if isinstance(inst, mybir.InstEventSemaphore) else 1
                if si is not None and len(si.on_wait) > cap:
                    waits = list(si.on_wait)
                    for w in waits[:-cap]:
                        nop = mybir.InstNoOp(name=f"{inst.name}-w{n}", ins=[], outs=[])
                        n += 1
                        nop.engine = inst.engine
                        nop.sync_info = mybir.SyncInfo(on_wait=[w], on_update=[])
                        out.append(nop)
                    inst.sync_info = mybir.SyncInfo(
                        on_wait=waits[-cap:], on_update=list(si.on_update)
                    )
                    changed = True
                out.append(inst)
            if changed:
                blk.instructions = out


def _build_nc(chunks):
    """Build the Bass program.  chunks[b] = per-seq 128-slot chunk count."""
    import concourse.bass as bass
    import concourse.mybir as mybir
    import concourse.tile as tile

    f32 = mybir.dt.float32
    kt_dt = getattr(mybir.dt, KT_MYBIR)
    vt_dt = getattr(mybir.dt, VT_MYBIR)
    q_dt = getattr(mybir.dt, Q_MYBIR)
    et_dt = getattr(mybir.dt, ET_MYBIR)
    total = sum(chunks)
    SPT = total * CHUNK

    nc = bass.Bass("TRN2", target_bir_lowering=False, debug=False, num_devices=NCORES)
    kt_d = nc.dram_tensor("kt", [D, SPT], kt_dt, kind="ExternalInput")
    vt_d = nc.dram_tensor("vt", [CHUNK, SPT], vt_dt, kind="ExternalInput")
    qt_d = nc.dram_tensor("qt", [D, B * G], q_dt, kind="ExternalInput")
    et_d = nc.dram_tensor("et", [CHUNK, total * G], et_dt, kind="ExternalOutput")
    out_d = nc.dram_tensor("out", [D, B * G], f32, kind="ExternalOutput")

    order = sorted(range(B), key=lambda i: (-chunks[i], i))
    groups = [order[i : i + GROUP] for i in range(0, B, GROUP)]

    with tile.TileContext(nc) as tc:
        with (
            tc.tile_pool(name="kv", bufs=1) as kv_pool,
            tc.tile_pool(name="small", bufs=1) as small_pool,
            tc.tile_pool(name="obuf", bufs=1) as ob_pool,
            tc.tile_pool(name="ps_s", bufs=4, space="PSUM") as ps_scores,
            tc.tile_pool(name="ps_o", bufs=4, space="PSUM") as ps_out,
        ):
            qt = small_pool.tile([D, B * G], q_dt)
            # tiny — goes on the otherwise-unused Pool SWDGE ring so it
            # doesn't delay the first kt trigger on the SP ring
            nc.gpsimd.dma_start(qt[:], qt_d[:])

            kts = {}  # b -> (tile, col offset)
            vts = {}
            koff = 0
            voff = 0
            loads = []  # (tile, src) in arrival order
            for gi, grp in enumerate(groups):
                gc = sum(chunks[b] for b in grp)
                vtg = kv_pool.tile(
                    [CHUNK, gc * CHUNK], vt_dt, tag=f"vtg{gi}", name=f"vtg{gi}"
                )
                vslice = vt_d[:, voff : voff + gc * CHUNK]
                off = 0
                for b in grp:
                    vts[b] = (vtg, off)
                    off += chunks[b] * CHUNK
                voff += gc * CHUNK

                if gi == 0:
                    # group 0's kt loads per-seq: shorter PE warm-up
                    for b in grp:
                        nb = chunks[b]
                        ktb = kv_pool.tile(
                            [D, nb * CHUNK], kt_dt, tag=f"kt{b}", name=f"kt{b}"
                        )
                        loads.append((ktb, kt_d[:, koff : koff + nb * CHUNK]))
                        kts[b] = (ktb, 0)
                        koff += nb * CHUNK
                else:
                    ktg = kv_pool.tile(
                        [D, gc * CHUNK], kt_dt, tag=f"ktg{gi}", name=f"ktg{gi}"
                    )
                    loads.append((ktg, kt_d[:, koff : koff + gc * CHUNK]))
                    off = 0
                    for b in grp:
                        kts[b] = (ktg, off)
                        off += chunks[b] * CHUNK
                    koff += gc * CHUNK
                loads.append((vtg, vslice))

            # Loads: bulk on the SP HWDGE ring in arrival order — a
            # single ring saturates HBM (descriptors fan out over all 16
            # DMA engines; measured 420 B/ns), and triggers past the
            # ring's descriptor capacity just block the SP queue where
            # nothing else runs.  The LAST four loads (the two smallest
            # groups) instead go up front on the ACT ring — it holds ~5
            # DMAs' descriptors, so 4 triggers never block the exps
            # queued after them, the data is SBUF-resident by ~15us, and
            # the SP ring's slow drain tail (its last ~2MB trickles as
            # the queue empties) stops gating the final sequences.
            for tile_, src in loads[:-4]:
                nc.sync.dma_start(tile_[:], src)
            for tile_, src in loads[-4:]:
                nc.scalar.dma_start(tile_[:], src)

            et_all = ob_pool.tile([CHUNK, total * G], et_dt)
            ob_all = ob_pool.tile([D, B * G], f32)
            goffs = {}
            goff = 0
            for b in order:
                goffs[b] = goff
                goff += chunks[b]

            def emit_scores(b):
                nb = chunks[b]
                kt, ko = kts[b]
                sc = ps_scores.tile([CHUNK, nb * G], f32, tag="sc", name=f"sc{b}")
                for cb in range(nb):
                    nc.tensor.matmul(
                        sc[:, cb * G : (cb + 1) * G],
                        kt[:, ko + cb * CHUNK : ko + (cb + 1) * CHUNK],
                        qt[:, b * G : (b + 1) * G],
                        start=True,
                        stop=True,
                    )
                et = et_all[:, goffs[b] * G : (goffs[b] + nb) * G]
                nc.scalar.activation(
                    et, sc[:], mybir.ActivationFunctionType.Exp, scale=SCALE
                )

            def emit_pv(b):
                nb = chunks[b]
                et = et_all[:, goffs[b] * G : (goffs[b] + nb) * G]
                vt, vo = vts[b]
                ot = ps_out.tile([D, G], f32, tag="ot", name=f"ot{b}")
                for cb in range(nb):
                    nc.tensor.matmul(
                        ot[:],
                        vt[:, vo + cb * CHUNK : vo + (cb + 1) * CHUNK],
                        et[:, cb * G : (cb + 1) * G],
                        start=(cb == 0),
                        stop=(cb == nb - 1),
                    )
                # un-normalized output; DVE just evacuates PSUM (the
                # softmax division happens on the host, which recomputes
                # den from the shipped expT with padded slots masked)
                nc.vector.tensor_copy(ob_all[:, b * G : (b + 1) * G], ot[:])

            # software-pipelined PE stream with deep lookahead: scores
            # run LOOKAHEAD sequences ahead of PVs so PV_b's exp_b wait
            # is always long satisfied, and each PV is emitted BEFORE the
            # next score block so a score blocked on its kt DMA never
            # holds a ready PV hostage in the in-order PE queue.
            # software-pipelined PE stream: scores run LOOKAHEAD
            # sequences ahead of PVs so PV_b's exp_b wait is always long
            # satisfied, and each PV is emitted BEFORE the next score
            # block so a score blocked on its kt DMA never holds a ready
            # PV hostage in the in-order PE queue
            corder = list(order)
            LOOKAHEAD = 5
            for i, b in enumerate(corder):
                if i >= LOOKAHEAD:
                    emit_pv(corder[i - LOOKAHEAD])
                emit_scores(b)
            for i in range(max(B - LOOKAHEAD, 0), B):
                emit_pv(corder[i])

            # stores last on the SP ring - they wait on compute, and a
            # store trigger issued mid-stream would block its engine queue
            # waiting for HWDGE ring space (the rings hold pending loads)
            nc.sync.dma_start(et_d[:], et_all[:])
            nc.sync.dma_start(out_d[:], ob_all[:])

    _split_multi_waits(nc)
    return nc


def kernel(q, k, v, k_cache, v_cache, slot_mapping, block_tables, context_lens):
    from concourse.bass_utils import run_bass_kernel_spmd

    global LAST_RESULT

    q = np.asarray(q, dtype=np.float32)
    k = np.asarray(k, dtype=np.float32)
    v = np.asarray(v, dtype=np.float32)
    k_cache = np.asarray(k_cache, dtype=np.float32)
    v_cache = np.asarray(v_cache, dtype=np.float32)
    slot_mapping = np.asarray(slot_mapping, dtype=np.int64)
    block_tables = np.asarray(block_tables, dtype=np.int64)
    context_lens = np.asarray(context_lens, dtype=np.int64)

    ctx = context_lens.astype(np.int64)
    chunks = tuple(int(max(1, -(-int(c) // CHUNK))) for c in ctx)
    total = sum(chunks)
    total_slots = total * CHUNK

    # Expanded slot index and validity mask for every sequence, concatenated
    # in device (descending-size) order.
    bt = np.maximum(block_tables, 0)
    order = sorted(range(B), key=lambda i: (-chunks[i], i))
    slots_parts = []
    valid_parts = []
    for b in order:
        sp = chunks[b] * CHUNK
        pos = np.arange(sp, dtype=np.int64)
        slots_parts.append(bt[b, pos // BLOCK_SIZE] * BLOCK_SIZE + pos % BLOCK_SIZE)
        valid_parts.append(pos < int(ctx[b]))
    slots_all = np.concatenate(slots_parts)
    valid_all = np.concatenate(valid_parts)

    # Where the freshly-scattered k/v rows land inside the gathered view.
    upd = []  # (gather-row index array, source batch index)
    for b2 in range(B):
        m = np.nonzero((slots_all == slot_mapping[b2]) & valid_all)[0]
        if m.size:
            upd.append((m, b2))

    if chunks not in _nc_cache:
        _nc_cache[chunks] = _build_nc(chunks)
    nc = _nc_cache[chunks]

    in_maps = []
    for c in range(NCORES):
        kg = k_cache[slots_all, c, :]
        vg = v_cache[slots_all, c, :]
        for m, b2 in upd:
            kg[m] = k[b2, c]
            vg[m] = v[b2, c]
        kg[~valid_all] = 0.0
        vg[~valid_all] = 0.0

        kt_h = np.ascontiguousarray(kg.T.astype(KT_NP))  # [128, SPT]
        vt_h = np.ascontiguousarray(
            vg.reshape(total, CHUNK, D)
            .transpose(1, 0, 2)
            .reshape(CHUNK, total * D)
            .astype(VT_NP)
        )
        qt_h = np.ascontiguousarray(
            q[:, c * G : (c + 1) * G, :].transpose(2, 0, 1).reshape(D, B * G).astype(Q_NP)
        )
        in_maps.append({"kt": kt_h, "vt": vt_h, "qt": qt_h})

    if TRACE:
        _install_ntff_shim()

    res = None
    for attempt in range(3):
        try:
            res = run_bass_kernel_spmd(
                nc, in_maps, core_ids=list(range(NCORES)), trace=TRACE
            )
            break
        except Exception:
            if attempt == 2:
                raise
    LAST_RESULT = res

    # host epilogue: den[g] = sum of expT over VALID slots, out = ot/den
    valid_by_chunk = valid_all.reshape(total, CHUNK)  # [chunk, slot]
    goffs = {}
    goff = 0
    for b in order:
        goffs[b] = goff
        goff += chunks[b]

    out = np.empty((B, H, D), dtype=np.float32)
    for c in range(NCORES):
        r = res.results[c]
        et_all = np.asarray(r["et"], dtype=np.float32)  # [128, total*G]
        ob_all = np.asarray(r["out"], dtype=np.float32)  # [128, B*G]
        et_c = et_all.reshape(CHUNK, total, G)  # [slot, chunk, g]
        for b in order:
            nb = chunks[b]
            go = goffs[b]
            m = valid_by_chunk[go : go + nb].T  # [slot, chunk]
            den = np.einsum("sc,scg->g", m.astype(np.float32), et_c[:, go : go + nb, :])
            out[b, c * G : (c + 1) * G, :] = (
                ob_all[:, b * G : (b + 1) * G] / den[None, :]
            ).T
    return np.ascontiguousarray(out, dtype=np.float32)


# revision 49
# speedup vs baseline: 1.0185x; 1.0185x over previous
"""Paged-attention GQA decode kernel for 8 Trainium2 NeuronCores.

Problem: B=16 sequences, H=32 query heads, KVH=8 KV heads (GQA group G=4),
D=128, paged KV cache of 65536 slots (block size 256, 16 blocks/seq,
max context 4096).

Sharding: tensor-parallel over KV heads — core c owns KV head c and the
4 query heads of its GQA group, for all 16 sequences.

Host-side prep (per core, plain numpy — this is the shard/relayout step):
  * scatter the new k/v rows into the cache view (reference step 1),
  * gather each sequence's context via its block table (reference step 2),
  * K transposed to [d, s] bf16 (the PE contracts over d); V slot-major
    per 128-slot chunk, [slot, d] fp8 e3m4 (4 mantissa bits; V error
    enters the output linearly and 1.3e-2 max-rel fits the 2e-2 gate,
    while fp8 K would not — exp amplifies score noise, measured 2.02e-2).
Rows past a sequence's context length are zeroed in V, so padded slots
contribute exactly 0 to the numerator; the host drops them from the
denominator sum (it knows the valid mask).

Device kernel (per core), per sequence, per 128-slot chunk:
  scoresT[s,g] = KT_chunk.T @ QT      (PE; 128-col bf16 stationary gets
                                       the automatic fast-weight-load)
  expT         = exp(scoresT * SCALE) (ACT, bf16 out; no max-subtraction
                                       — scores are ~N(0,1))
  ot[d,g]     += V_chunk.T @ expT     (PE; 128-col fp8 stationary → FWL,
                                       4 weights/cycle; only G=4 moving
                                       columns per chunk)
  ob[:, b]     = copy(ot)             (DVE evacuates PSUM)
and ships BOTH the un-normalized ot and the expT tensor to the host,
which computes den[g] = sum of valid expT rows and divides — softmax
normalization is a tiny host-side epilogue on [B,H] scalars.

DMA strategy: sequences are paired into 8 kt + 8 vt group-loads (a DMA's
HWDGE descriptor count is 128 either way, and the ring only holds ~3
DMAs' descriptors — bigger transfers keep more bytes in flight), all
issued up front: kt groups on the SP ring, vt groups on the ACT ring
ahead of the exps.  All KV tiles are SBUF-resident.  Measured bottleneck
is the KV-cache DMA traffic, as intended for this memory-bound regime.
"""

import ml_dtypes
import numpy as np

B, H, KVH, D = 16, 32, 8, 128
G = H // KVH  # 4
BLOCK_SIZE = 256
MAX_CTX = 4096
SCALE = 0.08838834764831845  # 1/sqrt(128)
NCORES = 8
CHUNK = 128
GROUP = 2  # sequences per load-DMA

KT_NP = ml_dtypes.bfloat16
VT_NP = ml_dtypes.float8_e3m4
Q_NP = ml_dtypes.bfloat16
ET_NP = ml_dtypes.bfloat16
KT_MYBIR = "bfloat16"
VT_MYBIR = "float8e3"
Q_MYBIR = "bfloat16"
ET_MYBIR = "bfloat16"

TRACE = False  # set by test harness to capture an NTFF profile
LAST_RESULT = None  # BassKernelResults of the most recent run (for the harness)

_nc_cache = {}


def _install_ntff_shim():
    """Register the NTFF profile hook concourse looks for under axon.

    The agent image's ``antenv`` lacks ``axon_hooks``; the ctypes hook
    implementation ships in ``trn_agent_boot`` — wire the two together.
    """
    import sys
    import types

    if "antenv.axon_hooks" in sys.modules:
        return
    try:
        import trn_agent_boot.trn_boot as tb

        hook = tb._ntff_profile_via_ctypes("/opt/axon/libaxon_pjrt.so")
    except Exception:
        return
    mod = types.ModuleType("antenv.axon_hooks")
    mod.get_axon_ntff_profile_hook = lambda: hook
    sys.modules["antenv.axon_hooks"] = mod


def _split_multi_waits(nc):
    """Legalize sync waits for this walrus build.

    The Tile scheduler attaches one wait per producer semaphore to an
    instruction (up to 4 here), but this walrus rejects more than 1 sync
    wait per instruction (2 on EventSemaphore).  Splitting the extras
    onto same-engine nops placed immediately before the instruction
    preserves semantics: engines execute their stream in order, so all
    waits still complete before the instruction runs.
    """
    import concourse.mybir as mybir

    n = 0
    for fn in nc.m.functions:
        for blk in fn.blocks:
            out = []
            changed = False
            for inst in blk.instructions:
                si = inst.sync_info
                cap = 2 if isinstance(inst, mybir.InstEventSemaphore) else 1
                if si is not None and len(si.on_wait) > cap:
                    waits = list(si.on_wait)
                    for w in waits[:-cap]:
                        nop = mybir.InstNoOp(name=f"{inst.name}-w{n}", ins=[], outs=[])
                        n += 1
                        nop.engine = inst.engine
                        nop.sync_info = mybir.SyncInfo(on_wait=[w], on_update=[])
                        out.append(nop)
                    inst.sync_info = mybir.SyncInfo(
                        on_wait=waits[-cap:], on_update=list(si.on_update)
                    )
                    changed = True
                out.append(inst)
            if changed:
                blk.instructions = out


def _build_nc(chunks):
    """Build the Bass program.  chunks[b] = per-seq 128-slot chunk count."""
    import concourse.bass as bass
    import concourse.mybir as mybir
    import concourse.tile as tile

    f32 = mybir.dt.float32
    kt_dt = getattr(mybir.dt, KT_MYBIR)
    vt_dt = getattr(mybir.dt, VT_MYBIR)
    q_dt = getattr(mybir.dt, Q_MYBIR)
    et_dt = getattr(mybir.dt, ET_MYBIR)
    total = sum(chunks)
    SPT = total * CHUNK

    nc = bass.Bass("TRN2", target_bir_lowering=False, debug=False, num_devices=NCORES)
    kt_d = nc.dram_tensor("kt", [D, SPT], kt_dt, kind="ExternalInput")
    vt_d = nc.dram_tensor("vt", [CHUNK, SPT], vt_dt, kind="ExternalInput")
    qt_d = nc.dram_tensor("qt", [D, B * G], q_dt, kind="ExternalInput")
    et_d = nc.dram_tensor("et", [CHUNK, total * G], et_dt, kind="ExternalOutput")
    out_d = nc.dram_tensor("out", [D, B * G], f32, kind="ExternalOutput")

    order = sorted(range(B), key=lambda i: (-chunks[i], i))
    groups = [order[i : i + GROUP] for i in range(0, B, GROUP)]

    with tile.TileContext(nc) as tc:
        with (
            tc.tile_pool(name="kv", bufs=1) as kv_pool,
            tc.tile_pool(name="small", bufs=1) as small_pool,
            tc.tile_pool(name="obuf", bufs=1) as ob_pool,
            tc.tile_pool(name="ps_s", bufs=6, space="PSUM") as ps_scores,
            tc.tile_pool(name="ps_o", bufs=2, space="PSUM") as ps_out,
        ):
            qt = small_pool.tile([D, B * G], q_dt)
            # tiny — goes on the otherwise-unused Pool SWDGE ring so it
            # doesn't delay the first kt trigger on the SP ring
            nc.gpsimd.dma_start(qt[:], qt_d[:])

            kts = {}  # b -> (tile, col offset)
            vts = {}
            koff = 0
            voff = 0
            loads = []  # (tile, src) in arrival order
            for gi, grp in enumerate(groups):
                gc = sum(chunks[b] for b in grp)
                vtg = kv_pool.tile(
                    [CHUNK, gc * CHUNK], vt_dt, tag=f"vtg{gi}", name=f"vtg{gi}"
                )
                vslice = vt_d[:, voff : voff + gc * CHUNK]
                off = 0
                for b in grp:
                    vts[b] = (vtg, off)
                    off += chunks[b] * CHUNK
                voff += gc * CHUNK

                if gi == 0:
                    # group 0's kt loads per-seq: shorter PE warm-up
                    for b in grp:
                        nb = chunks[b]
                        ktb = kv_pool.tile(
                            [D, nb * CHUNK], kt_dt, tag=f"kt{b}", name=f"kt{b}"
                        )
                        loads.append((ktb, kt_d[:, koff : koff + nb * CHUNK]))
                        kts[b] = (ktb, 0)
                        koff += nb * CHUNK
                else:
                    ktg = kv_pool.tile(
                        [D, gc * CHUNK], kt_dt, tag=f"ktg{gi}", name=f"ktg{gi}"
                    )
                    loads.append((ktg, kt_d[:, koff : koff + gc * CHUNK]))
                    off = 0
                    for b in grp:
                        kts[b] = (ktg, off)
                        off += chunks[b] * CHUNK
                    koff += gc * CHUNK
                loads.append((vtg, vslice))

            # Loads: bulk on the SP HWDGE ring in arrival order — a
            # single ring saturates HBM (descriptors fan out over all 16
            # DMA engines; measured 420 B/ns), and triggers past the
            # ring's descriptor capacity just block the SP queue where
            # nothing else runs.  The LAST four loads (the two smallest
            # groups) instead go up front on the ACT ring — it holds ~5
            # DMAs' descriptors, so 4 triggers never block the exps
            # queued after them, the data is SBUF-resident by ~15us, and
            # the SP ring's slow drain tail (its last ~2MB trickles as
            # the queue empties) stops gating the final sequences.
            for tile_, src in loads[:-4]:
                nc.sync.dma_start(tile_[:], src)
            for tile_, src in loads[-4:]:
                nc.scalar.dma_start(tile_[:], src)

            et_all = ob_pool.tile([CHUNK, total * G], et_dt)
            ob_all = ob_pool.tile([D, B * G], f32)
            goffs = {}
            goff = 0
            for b in order:
                goffs[b] = goff
                goff += chunks[b]

            def emit_scores(b):
                nb = chunks[b]
                kt, ko = kts[b]
                sc = ps_scores.tile([CHUNK, nb * G], f32, tag="sc", name=f"sc{b}")
                for cb in range(nb):
                    nc.tensor.matmul(
                        sc[:, cb * G : (cb + 1) * G],
                        kt[:, ko + cb * CHUNK : ko + (cb + 1) * CHUNK],
                        qt[:, b * G : (b + 1) * G],
                        start=True,
                        stop=True,
                    )
                et = et_all[:, goffs[b] * G : (goffs[b] + nb) * G]
                nc.scalar.activation(
                    et, sc[:], mybir.ActivationFunctionType.Exp, scale=SCALE
                )

            def emit_pv(b):
                nb = chunks[b]
                et = et_all[:, goffs[b] * G : (goffs[b] + nb) * G]
                vt, vo = vts[b]
                ot = ps_out.tile([D, G], f32, tag="ot", name=f"ot{b}")
                for cb in range(nb):
                    nc.tensor.matmul(
                        ot[:],
                        vt[:, vo + cb * CHUNK : vo + (cb + 1) * CHUNK],
                        et[:, cb * G : (cb + 1) * G],
                        start=(cb == 0),
                        stop=(cb == nb - 1),
                    )
                # un-normalized output; DVE just evacuates PSUM (the
                # softmax division happens on the host, which recomputes
                # den from the shipped expT with padded slots masked)
                nc.vector.tensor_copy(ob_all[:, b * G : (b + 1) * G], ot[:])

            # software-pipelined PE stream with deep lookahead: scores
            # run LOOKAHEAD sequences ahead of PVs so PV_b's exp_b wait
            # is always long satisfied, and each PV is emitted BEFORE the
            # next score block so a score blocked on its kt DMA never
            # holds a ready PV hostage in the in-order PE queue.
            # software-pipelined PE stream: scores run LOOKAHEAD
            # sequences ahead of PVs so PV_b's exp_b wait is always long
            # satisfied, and each PV is emitted BEFORE the next score
            # block so a score blocked on its kt DMA never holds a ready
            # PV hostage in the in-order PE queue
            corder = list(order)
            LOOKAHEAD = 8
            for i, b in enumerate(corder):
                if i >= LOOKAHEAD:
                    emit_pv(corder[i - LOOKAHEAD])
                emit_scores(b)
            for i in range(max(B - LOOKAHEAD, 0), B):
                emit_pv(corder[i])

            # stores last on the SP ring - they wait on compute, and a
            # store trigger issued mid-stream would block its engine queue
            # waiting for HWDGE ring space (the rings hold pending loads)
            nc.sync.dma_start(et_d[:], et_all[:])
            nc.sync.dma_start(out_d[:], ob_all[:])

    _split_multi_waits(nc)
    return nc


def kernel(q, k, v, k_cache, v_cache, slot_mapping, block_tables, context_lens):
    from concourse.bass_utils import run_bass_kernel_spmd

    global LAST_RESULT

    q = np.asarray(q, dtype=np.float32)
    k = np.asarray(k, dtype=np.float32)
    v = np.asarray(v, dtype=np.float32)
    k_cache = np.asarray(k_cache, dtype=np.float32)
    v_cache = np.asarray(v_cache, dtype=np.float32)
    slot_mapping = np.asarray(slot_mapping, dtype=np.int64)
    block_tables = np.asarray(block_tables, dtype=np.int64)
    context_lens = np.asarray(context_lens, dtype=np.int64)

    ctx = context_lens.astype(np.int64)
    chunks = tuple(int(max(1, -(-int(c) // CHUNK))) for c in ctx)
    total = sum(chunks)
    total_slots = total * CHUNK

    # Expanded slot index and validity mask for every sequence, concatenated
    # in device (descending-size) order.
    bt = np.maximum(block_tables, 0)
    order = sorted(range(B), key=lambda i: (-chunks[i], i))
    slots_parts = []
    valid_parts = []
    for b in order:
        sp = chunks[b] * CHUNK
        pos = np.arange(sp, dtype=np.int64)
        slots_parts.append(bt[b, pos // BLOCK_SIZE] * BLOCK_SIZE + pos % BLOCK_SIZE)
        valid_parts.append(pos < int(ctx[b]))
    slots_all = np.concatenate(slots_parts)
    valid_all = np.concatenate(valid_parts)

    # Where the freshly-scattered k/v rows land inside the gathered view.
    upd = []  # (gather-row index array, source batch index)
    for b2 in range(B):
        m = np.nonzero((slots_all == slot_mapping[b2]) & valid_all)[0]
        if m.size:
            upd.append((m, b2))

    if chunks not in _nc_cache:
        _nc_cache[chunks] = _build_nc(chunks)
    nc = _nc_cache[chunks]

    in_maps = []
    for c in range(NCORES):
        kg = k_cache[slots_all, c, :]
        vg = v_cache[slots_all, c, :]
        for m, b2 in upd:
            kg[m] = k[b2, c]
            vg[m] = v[b2, c]
        kg[~valid_all] = 0.0
        vg[~valid_all] = 0.0

        kt_h = np.ascontiguousarray(kg.T.astype(KT_NP))  # [128, SPT]
        vt_h = np.ascontiguousarray(
            vg.reshape(total, CHUNK, D)
            .transpose(1, 0, 2)
            .reshape(CHUNK, total * D)
            .astype(VT_NP)
        )
        qt_h = np.ascontiguousarray(
            q[:, c * G : (c + 1) * G, :].transpose(2, 0, 1).reshape(D, B * G).astype(Q_NP)
        )
        in_maps.append({"kt": kt_h, "vt": vt_h, "qt": qt_h})

    if TRACE:
        _install_ntff_shim()

    res = None
    for attempt in range(3):
        try:
            res = run_bass_kernel_spmd(
                nc, in_maps, core_ids=list(range(NCORES)), trace=TRACE
            )
            break
        except Exception:
            if attempt == 2:
                raise
    LAST_RESULT = res

    # host epilogue: den[g] = sum of expT over VALID slots, out = ot/den
    valid_by_chunk = valid_all.reshape(total, CHUNK)  # [chunk, slot]
    goffs = {}
    goff = 0
    for b in order:
        goffs[b] = goff
        goff += chunks[b]

    out = np.empty((B, H, D), dtype=np.float32)
    for c in range(NCORES):
        r = res.results[c]
        et_all = np.asarray(r["et"], dtype=np.float32)  # [128, total*G]
        ob_all = np.asarray(r["out"], dtype=np.float32)  # [128, B*G]
        et_c = et_all.reshape(CHUNK, total, G)  # [slot, chunk, g]
        for b in order:
            nb = chunks[b]
            go = goffs[b]
            m = valid_by_chunk[go : go + nb].T  # [slot, chunk]
            den = np.einsum("sc,scg->g", m.astype(np.float32), et_c[:, go : go + nb, :])
            out[b, c * G : (c + 1) * G, :] = (
                ob_all[:, b * G : (b + 1) * G] / den[None, :]
            ).T
    return np.ascontiguousarray(out, dtype=np.float32)


# revision 50
# speedup vs baseline: 1.0829x; 1.0632x over previous
"""Paged-attention GQA decode kernel for 8 Trainium2 NeuronCores.

Problem: B=16 sequences, H=32 query heads, KVH=8 KV heads (GQA group G=4),
D=128, paged KV cache of 65536 slots (block size 256, 16 blocks/seq,
max context 4096).

Sharding: tensor-parallel over KV heads — core c owns KV head c and the
4 query heads of its GQA group, for all 16 sequences.

Host-side prep (per core, plain numpy — this is the shard/relayout step):
  * scatter the new k/v rows into the cache view (reference step 1),
  * gather each sequence's context via its block table (reference step 2),
  * K transposed to [d, s] bf16 (the PE contracts over d); V slot-major
    per 128-slot chunk, [slot, d] fp8 e3m4 (4 mantissa bits; V error
    enters the output linearly and 1.3e-2 max-rel fits the 2e-2 gate,
    while fp8 K would not — exp amplifies score noise, measured 2.02e-2).
Rows past a sequence's context length are zeroed in V, so padded slots
contribute exactly 0 to the numerator; the host drops them from the
denominator sum (it knows the valid mask).

Device kernel (per core), per sequence, per 128-slot chunk:
  scoresT[s,g] = KT_chunk.T @ QT      (PE; 128-col bf16 stationary gets
                                       the automatic fast-weight-load)
  expT         = exp(scoresT * SCALE) (ACT, bf16 out; no max-subtraction
                                       — scores are ~N(0,1))
  ot[d,g]     += V_chunk.T @ expT     (PE; 128-col fp8 stationary → FWL,
                                       4 weights/cycle; only G=4 moving
                                       columns per chunk)
  ob[:, b]     = copy(ot)             (DVE evacuates PSUM)
and ships BOTH the un-normalized ot and the expT tensor to the host,
which computes den[g] = sum of valid expT rows and divides — softmax
normalization is a tiny host-side epilogue on [B,H] scalars.

DMA strategy: sequences are paired into 8 kt + 8 vt group-loads (a DMA's
HWDGE descriptor count is 128 either way, and the ring only holds ~3
DMAs' descriptors — bigger transfers keep more bytes in flight), all
issued up front: kt groups on the SP ring, vt groups on the ACT ring
ahead of the exps.  All KV tiles are SBUF-resident.  Measured bottleneck
is the KV-cache DMA traffic, as intended for this memory-bound regime.
"""

import ml_dtypes
import numpy as np

B, H, KVH, D = 16, 32, 8, 128
G = H // KVH  # 4
BLOCK_SIZE = 256
MAX_CTX = 4096
SCALE = 0.08838834764831845  # 1/sqrt(128)
NCORES = 8
CHUNK = 128
GROUP = 2  # sequences per load-DMA

KT_NP = ml_dtypes.bfloat16
VT_NP = ml_dtypes.float8_e3m4
Q_NP = ml_dtypes.bfloat16
ET_NP = ml_dtypes.bfloat16
KT_MYBIR = "bfloat16"
VT_MYBIR = "float8e3"
Q_MYBIR = "bfloat16"
ET_MYBIR = "bfloat16"

TRACE = False  # set by test harness to capture an NTFF profile
LAST_RESULT = None  # BassKernelResults of the most recent run (for the harness)

_nc_cache = {}


def _install_ntff_shim():
    """Register the NTFF profile hook concourse looks for under axon.

    The agent image's ``antenv`` lacks ``axon_hooks``; the ctypes hook
    implementation ships in ``trn_agent_boot`` — wire the two together.
    """
    import sys
    import types

    if "antenv.axon_hooks" in sys.modules:
        return
    try:
        import trn_agent_boot.trn_boot as tb

        hook = tb._ntff_profile_via_ctypes("/opt/axon/libaxon_pjrt.so")
    except Exception:
        return
    mod = types.ModuleType("antenv.axon_hooks")
    mod.get_axon_ntff_profile_hook = lambda: hook
    sys.modules["antenv.axon_hooks"] = mod


def _split_multi_waits(nc):
    """Legalize sync waits for this walrus build.

    The Tile scheduler attaches one wait per producer semaphore to an
    instruction (up to 4 here), but this walrus rejects more than 1 sync
    wait per instruction (2 on EventSemaphore).  Splitting the extras
    onto same-engine nops placed immediately before the instruction
    preserves semantics: engines execute their stream in order, so all
    waits still complete before the instruction runs.
    """
    import concourse.mybir as mybir

    n = 0
    for fn in nc.m.functions:
        for blk in fn.blocks:
            out = []
            changed = False
            for inst in blk.instructions:
                si = inst.sync_info
                cap = 2 if isinstance(inst, mybir.InstEventSemaphore) else 1
                if si is not None and len(si.on_wait) > cap:
                    waits = list(si.on_wait)
                    for w in waits[:-cap]:
                        nop = mybir.InstNoOp(name=f"{inst.name}-w{n}", ins=[], outs=[])
                        n += 1
                        nop.engine = inst.engine
                        nop.sync_info = mybir.SyncInfo(on_wait=[w], on_update=[])
                        out.append(nop)
                    inst.sync_info = mybir.SyncInfo(
                        on_wait=waits[-cap:], on_update=list(si.on_update)
                    )
                    changed = True
                out.append(inst)
            if changed:
                blk.instructions = out


def _build_nc(chunks):
    """Build the Bass program.  chunks[b] = per-seq 128-slot chunk count."""
    import concourse.bass as bass
    import concourse.mybir as mybir
    import concourse.tile as tile

    f32 = mybir.dt.float32
    kt_dt = getattr(mybir.dt, KT_MYBIR)
    vt_dt = getattr(mybir.dt, VT_MYBIR)
    q_dt = getattr(mybir.dt, Q_MYBIR)
    et_dt = getattr(mybir.dt, ET_MYBIR)
    total = sum(chunks)
    SPT = total * CHUNK

    nc = bass.Bass("TRN2", target_bir_lowering=False, debug=False, num_devices=NCORES)
    kt_d = nc.dram_tensor("kt", [D, SPT], kt_dt, kind="ExternalInput")
    vt_d = nc.dram_tensor("vt", [CHUNK, SPT], vt_dt, kind="ExternalInput")
    qt_d = nc.dram_tensor("qt", [D, B * G], q_dt, kind="ExternalInput")
    et_d = nc.dram_tensor("et", [CHUNK, total * G], et_dt, kind="ExternalOutput")
    out_d = nc.dram_tensor("out", [D, B * G], f32, kind="ExternalOutput")

    order = sorted(range(B), key=lambda i: (-chunks[i], i))
    groups = [order[i : i + GROUP] for i in range(0, B, GROUP)]

    with tile.TileContext(nc) as tc:
        with (
            tc.tile_pool(name="kv", bufs=1) as kv_pool,
            tc.tile_pool(name="small", bufs=1) as small_pool,
            tc.tile_pool(name="obuf", bufs=1) as ob_pool,
            tc.tile_pool(name="ps_s", bufs=6, space="PSUM") as ps_scores,
            tc.tile_pool(name="ps_o", bufs=2, space="PSUM") as ps_out,
        ):
            qt = small_pool.tile([D, B * G], q_dt)
            # tiny — goes on the otherwise-unused Pool SWDGE ring so it
            # doesn't delay the first kt trigger on the SP ring
            nc.gpsimd.dma_start(qt[:], qt_d[:])

            kts = {}  # b -> (tile, col offset)
            vts = {}
            koff = 0
            voff = 0
            loads = []  # (tile, src) in arrival order
            for gi, grp in enumerate(groups):
                gc = sum(chunks[b] for b in grp)
                vtg = kv_pool.tile(
                    [CHUNK, gc * CHUNK], vt_dt, tag=f"vtg{gi}", name=f"vtg{gi}"
                )
                vslice = vt_d[:, voff : voff + gc * CHUNK]
                off = 0
                for b in grp:
                    vts[b] = (vtg, off)
                    off += chunks[b] * CHUNK
                voff += gc * CHUNK

                if gi == 0:
                    # group 0's kt loads per-seq: shorter PE warm-up
                    for b in grp:
                        nb = chunks[b]
                        ktb = kv_pool.tile(
                            [D, nb * CHUNK], kt_dt, tag=f"kt{b}", name=f"kt{b}"
                        )
                        loads.append((ktb, kt_d[:, koff : koff + nb * CHUNK]))
                        kts[b] = (ktb, 0)
                        koff += nb * CHUNK
                else:
                    ktg = kv_pool.tile(
                        [D, gc * CHUNK], kt_dt, tag=f"ktg{gi}", name=f"ktg{gi}"
                    )
                    loads.append((ktg, kt_d[:, koff : koff + gc * CHUNK]))
                    off = 0
                    for b in grp:
                        kts[b] = (ktg, off)
                        off += chunks[b] * CHUNK
                    koff += gc * CHUNK
                loads.append((vtg, vslice))

            # Loads: bulk on the SP HWDGE ring in arrival order — a
            # single ring saturates HBM (descriptors fan out over all 16
            # DMA engines; measured 420 B/ns), and triggers past the
            # ring's descriptor capacity just block the SP queue where
            # nothing else runs.  The LAST four loads (the two smallest
            # groups) instead go up front on the ACT ring — it holds ~5
            # DMAs' descriptors, so 4 triggers never block the exps
            # queued after them, the data is SBUF-resident by ~15us, and
            # the SP ring's slow drain tail (its last ~2MB trickles as
            # the queue empties) stops gating the final sequences.
            for tile_, src in loads[:-4]:
                nc.sync.dma_start(tile_[:], src)
            for tile_, src in loads[-4:]:
                nc.scalar.dma_start(tile_[:], src)

            et_all = ob_pool.tile([CHUNK, total * G], et_dt)
            ob_all = ob_pool.tile([D, B * G], f32)
            goffs = {}
            goff = 0
            for b in order:
                goffs[b] = goff
                goff += chunks[b]

            def emit_scores(b):
                nb = chunks[b]
                kt, ko = kts[b]
                sc = ps_scores.tile([CHUNK, nb * G], f32, tag="sc", name=f"sc{b}")
                for cb in range(nb):
                    nc.tensor.matmul(
                        sc[:, cb * G : (cb + 1) * G],
                        kt[:, ko + cb * CHUNK : ko + (cb + 1) * CHUNK],
                        qt[:, b * G : (b + 1) * G],
                        start=True,
                        stop=True,
                    )
                et = et_all[:, goffs[b] * G : (goffs[b] + nb) * G]
                nc.scalar.activation(
                    et, sc[:], mybir.ActivationFunctionType.Exp, scale=SCALE
                )

            def emit_pv(b):
                nb = chunks[b]
                et = et_all[:, goffs[b] * G : (goffs[b] + nb) * G]
                vt, vo = vts[b]
                ot = ps_out.tile([D, G], f32, tag="ot", name=f"ot{b}")
                for cb in range(nb):
                    nc.tensor.matmul(
                        ot[:],
                        vt[:, vo + cb * CHUNK : vo + (cb + 1) * CHUNK],
                        et[:, cb * G : (cb + 1) * G],
                        start=(cb == 0),
                        stop=(cb == nb - 1),
                    )
                # un-normalized output; DVE just evacuates PSUM (the
                # softmax division happens on the host, which recomputes
                # den from the shipped expT with padded slots masked)
                nc.vector.tensor_copy(ob_all[:, b * G : (b + 1) * G], ot[:])

            # software-pipelined PE stream with deep lookahead: scores
            # run LOOKAHEAD sequences ahead of PVs so PV_b's exp_b wait
            # is always long satisfied, and each PV is emitted BEFORE the
            # next score block so a score blocked on its kt DMA never
            # holds a ready PV hostage in the in-order PE queue.
            # software-pipelined PE stream: scores run LOOKAHEAD
            # sequences ahead of PVs so PV_b's exp_b wait is always long
            # satisfied, and each PV is emitted BEFORE the next score
            # block so a score blocked on its kt DMA never holds a ready
            # PV hostage in the in-order PE queue
            corder = list(order)
            LOOKAHEAD = 5
            for i, b in enumerate(corder):
                if i >= LOOKAHEAD:
                    emit_pv(corder[i - LOOKAHEAD])
                emit_scores(b)
            for i in range(max(B - LOOKAHEAD, 0), B):
                emit_pv(corder[i])

            # stores last on the SP ring - they wait on compute, and a
            # store trigger issued mid-stream would block its engine queue
            # waiting for HWDGE ring space (the rings hold pending loads)
            nc.sync.dma_start(et_d[:], et_all[:])
            nc.sync.dma_start(out_d[:], ob_all[:])

    _split_multi_waits(nc)
    return nc


def kernel(q, k, v, k_cache, v_cache, slot_mapping, block_tables, context_lens):
    from concourse.bass_utils import run_bass_kernel_spmd

    global LAST_RESULT

    q = np.asarray(q, dtype=np.float32)
    k = np.asarray(k, dtype=np.float32)
    v = np.asarray(v, dtype=np.float32)
    k_cache = np.asarray(k_cache, dtype=np.float32)
    v_cache = np.asarray(v_cache, dtype=np.float32)
    slot_mapping = np.asarray(slot_mapping, dtype=np.int64)
    block_tables = np.asarray(block_tables, dtype=np.int64)
    context_lens = np.asarray(context_lens, dtype=np.int64)

    ctx = context_lens.astype(np.int64)
    chunks = tuple(int(max(1, -(-int(c) // CHUNK))) for c in ctx)
    total = sum(chunks)
    total_slots = total * CHUNK

    # Expanded slot index and validity mask for every sequence, concatenated
    # in device (descending-size) order.
    bt = np.maximum(block_tables, 0)
    order = sorted(range(B), key=lambda i: (-chunks[i], i))
    slots_parts = []
    valid_parts = []
    for b in order:
        sp = chunks[b] * CHUNK
        pos = np.arange(sp, dtype=np.int64)
        slots_parts.append(bt[b, pos // BLOCK_SIZE] * BLOCK_SIZE + pos % BLOCK_SIZE)
        valid_parts.append(pos < int(ctx[b]))
    slots_all = np.concatenate(slots_parts)
    valid_all = np.concatenate(valid_parts)

    # Where the freshly-scattered k/v rows land inside the gathered view.
    upd = []  # (gather-row index array, source batch index)
    for b2 in range(B):
        m = np.nonzero((slots_all == slot_mapping[b2]) & valid_all)[0]
        if m.size:
            upd.append((m, b2))

    if chunks not in _nc_cache:
        _nc_cache[chunks] = _build_nc(chunks)
    nc = _nc_cache[chunks]

    in_maps = []
    for c in range(NCORES):
        kg = k_cache[slots_all, c, :]
        vg = v_cache[slots_all, c, :]
        for m, b2 in upd:
            kg[m] = k[b2, c]
            vg[m] = v[b2, c]
        kg[~valid_all] = 0.0
        vg[~valid_all] = 0.0

        kt_h = np.ascontiguousarray(kg.T.astype(KT_NP))  # [128, SPT]
        vt_h = np.ascontiguousarray(
            vg.reshape(total, CHUNK, D)
            .transpose(1, 0, 2)
            .reshape(CHUNK, total * D)
            .astype(VT_NP)
        )
        qt_h = np.ascontiguousarray(
            q[:, c * G : (c + 1) * G, :].transpose(2, 0, 1).reshape(D, B * G).astype(Q_NP)
        )
        in_maps.append({"kt": kt_h, "vt": vt_h, "qt": qt_h})

    if TRACE:
        _install_ntff_shim()

    res = None
    for attempt in range(3):
        try:
            res = run_bass_kernel_spmd(
                nc, in_maps, core_ids=list(range(NCORES)), trace=TRACE
            )
            break
        except Exception:
            if attempt == 2:
                raise
    LAST_RESULT = res

    # host epilogue: den[g] = sum of expT over VALID slots, out = ot/den
    valid_by_chunk = valid_all.reshape(total, CHUNK)  # [chunk, slot]
    goffs = {}
    goff = 0
    for b in order:
        goffs[b] = goff
        goff += chunks[b]

    out = np.empty((B, H, D), dtype=np.float32)
    for c in range(NCORES):
        r = res.results[c]
        et_all = np.asarray(r["et"], dtype=np.float32)  # [128, total*G]
        ob_all = np.asarray(r["out"], dtype=np.float32)  # [128, B*G]
        et_c = et_all.reshape(CHUNK, total, G)  # [slot, chunk, g]
        for b in order:
            nb = chunks[b]
            go = goffs[b]
            m = valid_by_chunk[go : go + nb].T  # [slot, chunk]
            den = np.einsum("sc,scg->g", m.astype(np.float32), et_c[:, go : go + nb, :])
            out[b, c * G : (c + 1) * G, :] = (
                ob_all[:, b * G : (b + 1) * G] / den[None, :]
            ).T
    return np.ascontiguousarray(out, dtype=np.float32)
